# revision 1
# baseline (speedup 1.0000x reference)
"""BitLinear kernel for Trainium2, 8-core column-parallel.

Computes out = x @ (sign(W) * (weight_scale @ input_factor)).T
  x: [32, 8, 4096] f32, W: [11008, 4096] f32,
  weight_scale: [11008, 4] f32, input_factor: [4, 4096] f32
  -> out: [32, 8, 11008] f32

Sharding: column-parallel over out_features (11008 = 8 x 1376). Each core
gets its W / weight_scale row-shard plus replicated x / input_factor, and
produces out[:, core_slice]; host concatenates. No collectives.

Per-core dataflow (all on-device):
  - W ships as bf16 (sign-exact cast; only sign(W) is consumed) and is
    transposed by the DMA xbar on load: [128 i, 1376 o] strips, one per
    i-block. Halves the HBM stream and needs no PE transposes.
  - PE computes value strips value[i_blk, o_chunk] = f.T @ wsT (K=4 matmul)
  - ACT extracts s = sign(w) in {-1, 0, +1} via the Sign LUT (sign(+-0)=0,
    matching jnp.sign)
  - DVE multiplies w_signed = s * value (output cast to the matmul dtype)
  - PE main matmuls run as one dense burst per i-block:
    out[t, o] += xT_blk.T @ w_signed, accumulated in PSUM over all 32
    i-blocks (2 token-blocks x 3 banks + 2 value banks = 8 PSUM banks),
    then evacuated + DMA'd out.
Matmul operands use fp16 (11 mantissa bits, ~5e-4 rel err, full PE rate);
set BITLINEAR_PRECISION=f32 for exact-but-4x-slower fp32 matmuls.
"""

import os
import sys

if "/opt/trn_rl_repo" not in sys.path:
    sys.path.insert(0, "/opt/trn_rl_repo")

import numpy as np

# ---------------------------------------------------------------------------
# problem constants (hardcoded per the self-contained-kernel contract)
B, S, IN, OUT, R = 32, 8, 4096, 11008, 4
T = B * S               # 256 tokens
NCORES = 8
OS = OUT // NCORES      # 1376 out-features per core
P = 128
IC = 2048               # i-span per W DMA macro-tile
O_CHUNKS = [(0, 512), (512, 512), (1024, 352)]

# matmul precision mode:
#   "f32"  - plain fp32 matmuls (exact, ~1e-6 rel err) but TensorE runs
#            fp32 at 4 cycles/row -> PE-bound ~340us.
#   "f32r" - TF32-like fp32r (11 mantissa bits, 1 cycle/row at N>=256),
#            ~5e-4 rel err, ~3x faster. Well inside the 2e-2 gate.
PRECISION = os.environ.get("BITLINEAR_PRECISION", "f16")


def _install_tile_drain_patch():
    """This walrus build rejects >2 sync waits on one TPB_CTRL instruction;
    split the TileContext end-of-kernel drain into one drain per proc."""
    from concourse.tile import TileContext
    from concourse.vector_clock import ScopedClock
    from bass_rust import VectorClock

    if getattr(TileContext, "_drain_patch_installed", False):
        return

    def patched_drain_and_barrier(self, tick_clock, wait_clock):
        nc = self.nc
        gc = tick_clock.global_clock
        for i in range(27):
            v = gc[i]
            if v > 0:
                single = [0] * 27
                single[i] = v
                d = nc.sync.drain()
                wait_clock.add_sem_waits(
                    d.ins, ScopedClock({None: VectorClock(single)})
                )
        nc.all_engine_barrier()
        assert self.sems is not None
        popped = nc._tile_sem_poison_stack.pop()
        assert popped is self._sem_poison
        nc.clear_and_free_semaphores(list(self.sems.allocated().values()))
        nc.all_engine_barrier()

    TileContext._drain_and_barrier = patched_drain_and_barrier
    TileContext._drain_patch_installed = True


def _split_excess_waits(nc, max_waits=1):
    """This walrus build rejects instructions carrying more than ~2 sync
    waits. Move excess waits onto no-op instructions inserted immediately
    before the offender on the same engine (same semantics: the engine
    performs the same waits, in order, before executing the instruction)."""
    import concourse.mybir as mybir

    n_split = 0
    for fn in nc.m.functions:
        for bb in fn.blocks:
            insts = list(bb.instructions)
            new = []
            changed = False
            for inst in insts:
                si = inst.sync_info
                waits = list(si.on_wait) if si is not None else []
                if len(waits) > max_waits:
                    changed = True
                    n_split += 1
                    excess = waits[:-max_waits]
                    keep = waits[-max_waits:]
                    for i in range(0, len(excess), max_waits):
                        chunk = excess[i : i + max_waits]
                        nop = mybir.InstNoOp(
                            name=nc.get_next_instruction_name(),
                            sync_info=mybir.SyncInfo(
                                on_wait=chunk, on_update=[]
                            ),
                            bass_nofuse=True,
                            engine=inst.engine,
                        )
                        new.append(nop)
                    inst.sync_info = mybir.SyncInfo(
                        on_wait=keep, on_update=list(si.on_update)
                    )
                new.append(inst)
            if changed:
                bb.instructions = new
    return n_split


def build_nc():
    import concourse.bass as bass
    import concourse.mybir as mybir
    from concourse.bass import ts
    from concourse.masks import make_identity
    from concourse.tile import TileContext

    _install_tile_drain_patch()

    DT = mybir.dt.float32
    # fp16 keeps the same 11 explicit mantissa bits as fp32r (~2.4e-4 rel
    # err) but streams at full PE rate with fast weight loads; fp32r needs
    # every operand produced by a float32r-typed instruction (the rounding).
    MDT = {
        "f16": mybir.dt.float16,
        "f32r": mybir.dt.float32r,
        "f32": DT,
    }[PRECISION]
    nc = bass.Bass("TRN2", num_devices=NCORES)

    BF = mybir.dt.bfloat16
    # W ships as bf16 (sign-exact truncation of fp32 -- only its sign is
    # used) and is transposed by the DMA xbar on load, killing both the PE
    # transpose traffic and half the HBM stream. xT/wsT/f are pre-rounded
    # to the fp32r grid on the host and declared float32r so the DMA is a
    # valid fp32r producer for the matmuls.
    wbf_ext = nc.dram_tensor("wbf", [OS, IN], BF, kind="ExternalInput").ap()
    xT_ext = nc.dram_tensor("xT", [IN, T], MDT, kind="ExternalInput").ap()
    wsT_ext = nc.dram_tensor("wsT", [R, OS], MDT, kind="ExternalInput").ap()
    f_ext = nc.dram_tensor("f", [R, IN], MDT, kind="ExternalInput").ap()
    out_ext = nc.dram_tensor("out", [T, OS], DT, kind="ExternalOutput").ap()

    with TileContext(nc) as tc:
        with (
            tc.tile_pool(name="const", bufs=1) as cpool,
            tc.tile_pool(name="wtpool", bufs=4) as wtpool,
            tc.tile_pool(name="spool", bufs=4) as spool,
            tc.tile_pool(name="wsgpool", bufs=5) as wsgpool,
            tc.tile_pool(name="outsb", bufs=2) as outsb,
            tc.tile_pool(name="vpsum", bufs=2, space="PSUM") as vpool,
            tc.tile_pool(name="opsum", bufs=2, space="PSUM") as opool,
        ):
            # tiny f/wsT preloads go FIRST on the sync ring (ahead of the
            # W transposes) so the value matmuls unblock immediately; xT
            # rides the gpsimd SWDGE queue so the ACT ring is free to start
            # the Sign LUT work at t=0.
            f_sb = cpool.tile([R, IN], MDT)
            nc.sync.dma_start(f_sb[:, :], f_ext[:, :])
            wsT_sb = cpool.tile([R, OS], MDT)
            nc.sync.dma_start(wsT_sb[:, :], wsT_ext[:, :])

            # resident xT: [128, 32, 256], block ib holds xT[ib*128:(ib+1)*128, :]
            xT_sb = cpool.tile([P, IN // P, T], MDT)
            xT_view = xT_ext.rearrange("(a p) t -> p a t", p=P)
            for c0 in range(0, IN // P, 8):
                nc.gpsimd.dma_start(
                    xT_sb[:, c0 : c0 + 8], xT_view[:, c0 : c0 + 8]
                )

            n_iblk = IN // P  # 32
            # out[t, o] accumulates in PSUM across all 32 i-blocks:
            # 2 token-blocks x [128, 1376] fp32 = 2x3 banks, + 2 value
            # strips = 8 PSUM banks exactly.
            out_ps = [
                opool.tile([P, OS], DT, tag="out_ps", name=f"out_ps{tb}")
                for tb in range(2)
            ]
            for ib in range(n_iblk):
                first = ib == 0
                last = ib == n_iblk - 1
                # W strip [i_blk=128, all 1376 out-features], transposed by
                # the DMA xbar straight out of DRAM.
                wT_bf = wtpool.tile([P, OS], BF, tag="wT_bf", name="wT_bf")
                nc.sync.dma_start_transpose(
                    wT_bf[:, :], wbf_ext[:, ts(ib, P)]
                )
                # first produce all three signed-weight strips, then fire
                # the six main matmuls as one dense PE burst
                wsgs = []
                for (o0, No) in O_CHUNKS:
                    value_ps = vpool.tile(
                        [P, No], DT, tag="value_ps", name="value_ps"
                    )
                    nc.tensor.matmul(
                        value_ps,
                        f_sb[:, ts(ib, P)],
                        wsT_sb[:, o0 : o0 + No],
                        start=True,
                        stop=True,
                    )
                    # s = sign(w) in {-1, 0, +1} via ACT's Sign LUT
                    # (sign(+-0) = 0, matching jnp.sign), then one DVE
                    # multiply: w_signed = s * value; the DVE output cast
                    # doubles as the precision rounding.
                    s_sb = spool.tile([P, No], DT, tag="s_sb", name="s_sb")
                    nc.scalar.activation(
                        s_sb,
                        wT_bf[:, o0 : o0 + No],
                        mybir.ActivationFunctionType.Sign,
                    )
                    wsg_sb = wsgpool.tile(
                        [P, No], MDT, tag="wsg_sb", name="wsg_sb"
                    )
                    nc.vector.tensor_mul(wsg_sb, s_sb, value_ps)
                    wsgs.append(wsg_sb)
                for tb in range(2):
                    for (o0, No), wsg_sb in zip(O_CHUNKS, wsgs):
                        nc.tensor.matmul(
                            out_ps[tb][:, o0 : o0 + No],
                            xT_sb[:, ib, ts(tb, P)],
                            wsg_sb,
                            start=first,
                            stop=last,
                        )
            for tb in range(2):
                o_sb = outsb.tile([P, OS], DT, tag="o_sb", name="o_sb")
                nc.scalar.copy(o_sb, out_ps[tb])
                nc.scalar.dma_start(out_ext[ts(tb, P), :], o_sb)

    _split_excess_waits(nc)
    return nc


_NC_CACHE = None


def round_f32r(a):
    """Cast a matmul operand to the active precision grid: np.float16 for
    f16 mode; fp32 bits rounded to 11 explicit mantissa bits (RNE) for
    f32r mode -- what the on-device fp32r cast would produce."""
    if PRECISION == "f16":
        return np.ascontiguousarray(a, dtype=np.float32).astype(np.float16)
    if PRECISION != "f32r":
        return a
    bits = np.ascontiguousarray(a, dtype=np.float32).view(np.uint32)
    drop = 12
    q = np.uint32(1 << drop)
    lsb = (bits >> drop) & 1
    rounded = (bits + (q >> 1) - 1 + lsb) & ~(q - np.uint32(1))
    return rounded.view(np.float32)


def make_in_maps(x, weight, weight_scale, input_factor):
    import ml_dtypes

    xT = round_f32r(
        np.ascontiguousarray(x.reshape(T, IN).T.astype(np.float32))
    )
    f = round_f32r(np.ascontiguousarray(input_factor.astype(np.float32)))
    # only sign(weight) is used downstream; the bf16 cast preserves it
    # exactly (including +-0 -> sign 0)
    wbf = np.ascontiguousarray(weight.astype(ml_dtypes.bfloat16))
    in_maps = []
    for c in range(NCORES):
        sl = slice(c * OS, (c + 1) * OS)
        in_maps.append(
            {
                "wbf": wbf[sl],
                "xT": xT,
                "wsT": round_f32r(
                    np.ascontiguousarray(
                        weight_scale[sl].T.astype(np.float32)
                    )
                ),
                "f": f,
            }
        )
    return in_maps


def gather_out(results):
    outs = [results[c]["out"] for c in range(NCORES)]
    full = np.concatenate(outs, axis=1)  # [T, OUT]
    return np.ascontiguousarray(full.reshape(B, S, OUT).astype(np.float32))


def kernel(x, weight, weight_scale, input_factor):
    global _NC_CACHE
    from concourse.bass_utils import run_bass_kernel_spmd

    if _NC_CACHE is None:
        _NC_CACHE = build_nc()
    nc = _NC_CACHE

    in_maps = make_in_maps(x, weight, weight_scale, input_factor)
    res = run_bass_kernel_spmd(nc, in_maps, core_ids=list(range(NCORES)))
    return gather_out(res.results)


if __name__ == "__main__":
    # quick self-run with random data
    rng = np.random.default_rng(0)
    x = rng.standard_normal((B, S, IN), dtype=np.float32)
    w = rng.standard_normal((OUT, IN), dtype=np.float32)
    ws = rng.standard_normal((OUT, R), dtype=np.float32)
    f = rng.standard_normal((R, IN), dtype=np.float32)
    out = kernel(x=x, weight=w, weight_scale=ws, input_factor=f)
    wv = ws @ f
    expected = np.einsum("bsi,oi->bso", x, np.sign(w) * wv)
    rel = np.abs(out - expected).max() / np.abs(expected).max()
    print("rel err:", rel)



# revision 2
# speedup vs baseline: 2.2113x; 2.2113x over previous
"""BitLinear kernel for Trainium2, 8-core column-parallel.

Computes out = x @ (sign(W) * (weight_scale @ input_factor)).T
  x: [32, 8, 4096] f32, W: [11008, 4096] f32,
  weight_scale: [11008, 4] f32, input_factor: [4, 4096] f32
  -> out: [32, 8, 11008] f32

Sharding: column-parallel over out_features (11008 = 8 x 1376). Each core
gets its effective-weight row-shard plus replicated x; host concatenates.
No collectives.

The effective weight w_eff = sign(W) * (weight_scale @ input_factor) is
formed on the host (cheap: one rank-4 expansion + sign multiply), cast to
fp16 (~5e-4 rel err, well inside the 2e-2 gate) and shipped PRE-TRANSPOSED
as [in, out_shard] so every device DMA is a contiguous line-rate stream --
no DMA transpose, no on-device sign/value work.

Per-core dataflow:
  - wT [4096, 1376] fp16 streams as 16 macro-tiles [128, 2, 1376]
    (704 KB each, contiguous) on the sync HWDGE ring.
  - xT [4096, 256] fp16 is resident in SBUF, loaded as 4 chunks on the
    scalar HWDGE ring so the first matmul unblocks at ~2 us.
  - PE: for each of 32 K-blocks, 2 token-blocks x 3 N-chunks (512/512/352)
    accumulate out[t, o] += xT_blk.T @ wT_blk in PSUM (6 banks), one dense
    matmul burst -> PE stays at stream rate / HAM-warm throughout.
  - Epilogue: ACT copies PSUM -> SBUF fp16, DMA out; host upcasts to f32.
"""

import sys

if "/opt/trn_rl_repo" not in sys.path:
    sys.path.insert(0, "/opt/trn_rl_repo")

import numpy as np

# ---------------------------------------------------------------------------
# problem constants (hardcoded per the self-contained-kernel contract)
B, S, IN, OUT, R = 32, 8, 4096, 11008, 4
T = B * S               # 256 tokens
NCORES = 8
OS = OUT // NCORES      # 1376 out-features per core
P = 128
NBLK = IN // P          # 32 K-blocks
WJ = 2                  # K-blocks per W macro-tile DMA
O_CHUNKS = [(0, 512), (512, 512), (1024, 352)]


def _install_tile_drain_patch():
    """This walrus build rejects >2 sync waits on one TPB_CTRL instruction;
    split the TileContext end-of-kernel drain into one drain per proc."""
    from concourse.tile import TileContext
    from concourse.vector_clock import ScopedClock
    from bass_rust import VectorClock

    if getattr(TileContext, "_drain_patch_installed", False):
        return

    def patched_drain_and_barrier(self, tick_clock, wait_clock):
        nc = self.nc
        gc = tick_clock.global_clock
        for i in range(27):
            v = gc[i]
            if v > 0:
                single = [0] * 27
                single[i] = v
                d = nc.sync.drain()
                wait_clock.add_sem_waits(
                    d.ins, ScopedClock({None: VectorClock(single)})
                )
        nc.all_engine_barrier()
        assert self.sems is not None
        popped = nc._tile_sem_poison_stack.pop()
        assert popped is self._sem_poison
        nc.clear_and_free_semaphores(list(self.sems.allocated().values()))
        nc.all_engine_barrier()

    TileContext._drain_and_barrier = patched_drain_and_barrier
    TileContext._drain_patch_installed = True


def _split_excess_waits(nc, max_waits=1):
    """This walrus build rejects instructions carrying more than ~2 sync
    waits. Move excess waits onto no-op instructions inserted immediately
    before the offender on the same engine (same semantics: the engine
    performs the same waits, in order, before executing the instruction)."""
    import concourse.mybir as mybir

    n_split = 0
    for fn in nc.m.functions:
        for bb in fn.blocks:
            insts = list(bb.instructions)
            new = []
            changed = False
            for inst in insts:
                si = inst.sync_info
                waits = list(si.on_wait) if si is not None else []
                if len(waits) > max_waits:
                    changed = True
                    n_split += 1
                    excess = waits[:-max_waits]
                    keep = waits[-max_waits:]
                    for i in range(0, len(excess), max_waits):
                        chunk = excess[i : i + max_waits]
                        nop = mybir.InstNoOp(
                            name=nc.get_next_instruction_name(),
                            sync_info=mybir.SyncInfo(
                                on_wait=chunk, on_update=[]
                            ),
                            bass_nofuse=True,
                            engine=inst.engine,
                        )
                        new.append(nop)
                    inst.sync_info = mybir.SyncInfo(
                        on_wait=keep, on_update=list(si.on_update)
                    )
                new.append(inst)
            if changed:
                bb.instructions = new
    return n_split


def build_nc():
    import concourse.bass as bass
    import concourse.mybir as mybir
    from concourse.bass import ts
    from concourse.tile import TileContext

    _install_tile_drain_patch()

    F16 = mybir.dt.float16
    F32 = mybir.dt.float32
    nc = bass.Bass("TRN2", num_devices=NCORES)

    wT_ext = nc.dram_tensor("wT", [IN, OS], F16, kind="ExternalInput").ap()
    xT_ext = nc.dram_tensor("xT", [IN, T], F16, kind="ExternalInput").ap()
    out_ext = nc.dram_tensor("out", [T, OS], F16, kind="ExternalOutput").ap()

    with TileContext(nc) as tc:
        with (
            tc.tile_pool(name="const", bufs=1) as cpool,
            tc.tile_pool(name="wpool", bufs=4) as wpool,
            tc.tile_pool(name="outsb", bufs=2) as outsb,
            tc.tile_pool(name="opsum", bufs=2, space="PSUM") as opool,
        ):
            # resident xT: [128, 32, 256]; 4 chunks on the scalar HWDGE
            # ring (sync carries the W stream) so chunk 0 lands early.
            xT_sb = cpool.tile([P, NBLK, T], F16)
            xT_view = xT_ext.rearrange("(a p) t -> p a t", p=P)
            for c0 in range(0, NBLK, 8):
                nc.scalar.dma_start(
                    xT_sb[:, c0 : c0 + 8], xT_view[:, c0 : c0 + 8]
                )

            # W macro-tiles: [128, 2, 1376] fp16 (704 KB, contiguous rows)
            wT_view = wT_ext.rearrange("(k j p) o -> p k j o", j=WJ, p=P)

            # out[t, o] accumulates in PSUM across all 32 K-blocks:
            # 2 token-blocks x [128, 1376] fp32 = 3 banks each.
            out_ps = [
                opool.tile([P, OS], F32, tag="out_ps", name=f"out_ps{tb}")
                for tb in range(2)
            ]
            for k in range(NBLK // WJ):
                w_sb = wpool.tile([P, WJ, OS], F16, tag="w_sb", name="w_sb")
                nc.sync.dma_start(w_sb[:, :, :], wT_view[:, k])
                for j in range(WJ):
                    ib = k * WJ + j
                    first = ib == 0
                    last = ib == NBLK - 1
                    for tb in range(2):
                        for (o0, No) in O_CHUNKS:
                            nc.tensor.matmul(
                                out_ps[tb][:, o0 : o0 + No],
                                xT_sb[:, ib, ts(tb, P)],
                                w_sb[:, j, o0 : o0 + No],
                                start=first,
                                stop=last,
                            )
            for tb in range(2):
                o_sb = outsb.tile([P, OS], F16, tag="o_sb", name="o_sb")
                nc.scalar.copy(o_sb, out_ps[tb])
                nc.scalar.dma_start(out_ext[ts(tb, P), :], o_sb)

    _split_excess_waits(nc)
    return nc


_NC_CACHE = None


def make_in_maps(x, weight, weight_scale, input_factor):
    # effective weight on host: rank-4 expansion + sign, fp16, transposed
    w_eff = np.sign(weight, dtype=np.float32) * (
        weight_scale.astype(np.float32) @ input_factor.astype(np.float32)
    )
    w_effT = np.ascontiguousarray(w_eff.T.astype(np.float16))  # [IN, OUT]
    xT = np.ascontiguousarray(
        x.reshape(T, IN).T.astype(np.float16)
    )  # [IN, T]
    in_maps = []
    for c in range(NCORES):
        sl = slice(c * OS, (c + 1) * OS)
        in_maps.append(
            {
                "wT": np.ascontiguousarray(w_effT[:, sl]),
                "xT": xT,
            }
        )
    return in_maps


def gather_out(results):
    outs = [results[c]["out"] for c in range(NCORES)]
    full = np.concatenate(outs, axis=1)  # [T, OUT] fp16
    return np.ascontiguousarray(full.reshape(B, S, OUT).astype(np.float32))


def kernel(x, weight, weight_scale, input_factor):
    global _NC_CACHE
    from concourse.bass_utils import run_bass_kernel_spmd

    if _NC_CACHE is None:
        _NC_CACHE = build_nc()
    nc = _NC_CACHE

    in_maps = make_in_maps(x, weight, weight_scale, input_factor)
    res = run_bass_kernel_spmd(nc, in_maps, core_ids=list(range(NCORES)))
    return gather_out(res.results)


if __name__ == "__main__":
    # quick self-run with random data
    rng = np.random.default_rng(0)
    x = rng.standard_normal((B, S, IN), dtype=np.float32)
    w = rng.standard_normal((OUT, IN), dtype=np.float32)
    ws = rng.standard_normal((OUT, R), dtype=np.float32)
    f = rng.standard_normal((R, IN), dtype=np.float32)
    out = kernel(x=x, weight=w, weight_scale=ws, input_factor=f)
    wv = ws @ f
    expected = np.einsum("bsi,oi->bso", x, np.sign(w) * wv)
    rel = np.abs(out - expected).max() / np.abs(expected).max()
    print("rel err:", rel)


# revision 3
# speedup vs baseline: 2.5603x; 1.1578x over previous
"""BitLinear kernel for Trainium2, 8-core column-parallel.

Computes out = x @ (sign(W) * (weight_scale @ input_factor)).T
  x: [32, 8, 4096] f32, W: [11008, 4096] f32,
  weight_scale: [11008, 4] f32, input_factor: [4, 4096] f32
  -> out: [32, 8, 11008] f32

Sharding: column-parallel over out_features (11008 = 8 x 1376). Each core
gets its effective-weight row-shard plus replicated x; host concatenates.
No collectives.

The effective weight w_eff = sign(W) * (weight_scale @ input_factor) is
formed on the host (one rank-4 expansion + sign multiply), cast to fp16
(~5e-4 rel err, well inside the 2e-2 gate) and shipped PRE-TRANSPOSED and
partition-major, so every device DMA descriptor is a contiguous 2.75-5.5 KB
line -- no DMA transpose, no on-device sign/value work, line-rate HBM.

Per-core dataflow:
  - wT [128, 32, 1376] fp16 streams as macro-tiles on the sync HWDGE ring
    (first two macros are single K-blocks so the first matmul unblocks
    early, then 2-block macros).
  - xT [128, 32, 256] fp16 resident in SBUF, 4 chunks on the scalar ring.
  - A short burst of dummy warm-up matmuls on a zeroed tile keeps the PE
    busy from t~=7.5us so HAM un-throttles (1.2 -> 2.4 GHz) before the
    first real matmul's data lands.
  - PE: for each of 32 K-blocks, 2 token-blocks x 3 N-chunks (512/512/352)
    accumulate out[t, o] += xT_blk.T @ wT_blk in PSUM (6 banks).
    Redundant per-chunk LDWEIGHTS are deduped post-build (one stationary
    load per (K-block, token-block) instead of three).
  - Epilogue: per-chunk ACT copies PSUM -> SBUF fp16, DMA out on the sync
    ring; host upcasts to f32.
"""

import sys

if "/opt/trn_rl_repo" not in sys.path:
    sys.path.insert(0, "/opt/trn_rl_repo")

import numpy as np

# ---------------------------------------------------------------------------
# problem constants (hardcoded per the self-contained-kernel contract)
B, S, IN, OUT, R = 32, 8, 4096, 11008, 4
T = B * S               # 256 tokens
NCORES = 8
OS = OUT // NCORES      # 1376 out-features per core
P = 128
NBLK = IN // P          # 32 K-blocks
O_CHUNKS = [(0, 512), (512, 512), (1024, 352)]
W_MACROS = [1, 1] + [2] * 15        # K-blocks per W DMA (sum = 32)
X_CHUNKS = [4, 4, 8, 16]            # xT blocks per DMA (sum = 32)
N_WARMUP = 14                       # dummy PE matmuls to warm the HAM


def _install_tile_drain_patch():
    """This walrus build rejects >2 sync waits on one TPB_CTRL instruction;
    split the TileContext end-of-kernel drain into one drain per proc."""
    from concourse.tile import TileContext
    from concourse.vector_clock import ScopedClock
    from bass_rust import VectorClock

    if getattr(TileContext, "_drain_patch_installed", False):
        return

    def patched_drain_and_barrier(self, tick_clock, wait_clock):
        nc = self.nc
        gc = tick_clock.global_clock
        for i in range(27):
            v = gc[i]
            if v > 0:
                single = [0] * 27
                single[i] = v
                d = nc.sync.drain()
                wait_clock.add_sem_waits(
                    d.ins, ScopedClock({None: VectorClock(single)})
                )
        nc.all_engine_barrier()
        assert self.sems is not None
        popped = nc._tile_sem_poison_stack.pop()
        assert popped is self._sem_poison
        nc.clear_and_free_semaphores(list(self.sems.allocated().values()))
        nc.all_engine_barrier()

    TileContext._drain_and_barrier = patched_drain_and_barrier
    TileContext._drain_patch_installed = True


def _split_excess_waits(nc, max_waits=1):
    """This walrus build rejects instructions carrying more than ~2 sync
    waits. Move excess waits onto no-op instructions inserted immediately
    before the offender on the same engine (same semantics: the engine
    performs the same waits, in order, before executing the instruction)."""
    import concourse.mybir as mybir

    n_split = 0
    for fn in nc.m.functions:
        for bb in fn.blocks:
            insts = list(bb.instructions)
            new = []
            changed = False
            for inst in insts:
                si = inst.sync_info
                waits = list(si.on_wait) if si is not None else []
                if len(waits) > max_waits:
                    changed = True
                    n_split += 1
                    excess = waits[:-max_waits]
                    keep = waits[-max_waits:]
                    for i in range(0, len(excess), max_waits):
                        chunk = excess[i : i + max_waits]
                        nop = mybir.InstNoOp(
                            name=nc.get_next_instruction_name(),
                            sync_info=mybir.SyncInfo(
                                on_wait=chunk, on_update=[]
                            ),
                            bass_nofuse=True,
                            engine=inst.engine,
                        )
                        new.append(nop)
                    inst.sync_info = mybir.SyncInfo(
                        on_wait=keep, on_update=list(si.on_update)
                    )
                new.append(inst)
            if changed:
                bb.instructions = new
    return n_split


def _dedup_ldweights(nc):
    """Legalization splits every InstMatmult into LDWEIGHTS+MATMUL, so a
    stationary operand reused by consecutive matmuls (our 3 N-chunks per
    token-block) is reloaded each time. Drop an InstLdweights whose
    signature (access pattern, perf mode, tile position/size) matches the
    previous one on the PE queue with only matmuls/semaphores in between;
    its waits/updates migrate to the next PE instruction."""
    n_removed = 0
    passthrough = {"InstMatmult", "InstNoOp", "InstEventSemaphore", "InstDrain"}
    for fn in nc.m.functions:
        for bb in fn.blocks:
            insts = list(bb.instructions)
            new = []
            last_sig = None
            pend_waits = []
            pend_updates = []
            changed = False
            for inst in insts:
                tn = type(inst).__name__
                is_pe = getattr(inst, "engine", None) == nc.tensor.engine
                if tn == "InstLdweights" and is_pe:
                    ap = inst.ins[0]
                    sig = (
                        ap.concise(),
                        getattr(ap, "offset", None),
                        str(inst.perf_mode),
                        str(inst.is_transpose),
                        str(inst.tile_position),
                        str(inst.tile_size),
                    )
                    if sig == last_sig:
                        si = inst.sync_info
                        if si is not None:
                            pend_waits.extend(si.on_wait)
                            pend_updates.extend(si.on_update)
                        n_removed += 1
                        changed = True
                        continue
                    last_sig = sig
                elif is_pe and tn not in passthrough:
                    last_sig = None
                if is_pe and (pend_waits or pend_updates):
                    import concourse.mybir as mybir

                    si = inst.sync_info
                    waits = list(si.on_wait) if si is not None else []
                    updates = list(si.on_update) if si is not None else []
                    seen = {
                        (w.sync_type, w.id, w.wait_mode, w.wait_value)
                        for w in waits
                    }
                    for w in pend_waits:
                        k = (w.sync_type, w.id, w.wait_mode, w.wait_value)
                        if k not in seen:
                            seen.add(k)
                            waits.append(w)
                    updates.extend(pend_updates)
                    inst.sync_info = mybir.SyncInfo(
                        on_wait=waits, on_update=updates
                    )
                    pend_waits = []
                    pend_updates = []
                new.append(inst)
            if changed:
                bb.instructions = new
    return n_removed


def build_nc():
    import concourse.bass as bass
    import concourse.mybir as mybir
    from concourse.bass import ts
    from concourse.tile import TileContext

    _install_tile_drain_patch()

    F16 = mybir.dt.float16
    F32 = mybir.dt.float32
    nc = bass.Bass("TRN2", num_devices=NCORES)

    wT_ext = nc.dram_tensor(
        "wT", [P, NBLK * OS], F16, kind="ExternalInput"
    ).ap()
    xT_ext = nc.dram_tensor(
        "xT", [P, NBLK * T], F16, kind="ExternalInput"
    ).ap()
    out_ext = nc.dram_tensor("out", [T, OS], F16, kind="ExternalOutput").ap()

    with TileContext(nc) as tc:
        with (
            tc.tile_pool(name="const", bufs=1) as cpool,
            tc.tile_pool(name="wpool", bufs=4) as wpool,
            tc.tile_pool(name="outsb", bufs=6) as outsb,
            tc.tile_pool(name="opsum", bufs=2, space="PSUM") as opool,
            tc.tile_pool(name="wupsum", bufs=1, space="PSUM") as wupool,
        ):
            # --- PE warm-up: dummy matmuls on a zeroed tile, no DMA deps,
            # so the HAM clock gate opens before real data lands.
            wu_a = cpool.tile([P, P], F16)
            wu_b = cpool.tile([P, 512], F16)
            nc.vector.memset(wu_a[:, :], 0)
            nc.vector.memset(wu_b[:, :], 0)
            wu_ps = wupool.tile([P, 512], F32)
            for _ in range(N_WARMUP):
                nc.tensor.matmul(wu_ps, wu_a, wu_b, start=True, stop=True)

            # --- resident xT: [128, 32, 256], staged chunks on the scalar
            # HWDGE ring (sync carries the W stream).
            xT_sb = cpool.tile([P, NBLK, T], F16)
            xT_view = xT_ext.rearrange("p (a t) -> p a t", t=T)
            a0 = 0
            for al in X_CHUNKS:
                nc.scalar.dma_start(
                    xT_sb[:, a0 : a0 + al], xT_view[:, a0 : a0 + al]
                )
                a0 += al

            # --- W stream: partition-major macro-tiles, 2.75-5.5 KB
            # contiguous per partition per macro.
            wT_view = wT_ext.rearrange("p (k o) -> p k o", o=OS)
            out_ps = [
                opool.tile([P, OS], F32, tag="out_ps", name=f"out_ps{tb}")
                for tb in range(2)
            ]
            k0 = 0
            for kl in W_MACROS:
                w_sb = wpool.tile(
                    [P, kl, OS], F16, tag=f"w_sb{kl}", name="w_sb"
                )
                nc.sync.dma_start(w_sb[:, :, :], wT_view[:, k0 : k0 + kl])
                for j in range(kl):
                    ib = k0 + j
                    first = ib == 0
                    last = ib == NBLK - 1
                    for tb in range(2):
                        for (o0, No) in O_CHUNKS:
                            nc.tensor.matmul(
                                out_ps[tb][:, o0 : o0 + No],
                                xT_sb[:, ib, ts(tb, P)],
                                w_sb[:, j, o0 : o0 + No],
                                start=first,
                                stop=last,
                            )
                k0 += kl

            # --- epilogue: per-chunk PSUM->SBUF fp16 copy (ACT), DMA out
            # on the now-idle sync ring.
            for tb in range(2):
                for (o0, No) in O_CHUNKS:
                    o_sb = outsb.tile([P, No], F16, tag="o_sb", name="o_sb")
                    nc.scalar.copy(o_sb, out_ps[tb][:, o0 : o0 + No])
                    nc.sync.dma_start(
                        out_ext[ts(tb, P), o0 : o0 + No], o_sb
                    )

    _dedup_ldweights(nc)
    _split_excess_waits(nc)
    return nc


_NC_CACHE = None


def make_in_maps(x, weight, weight_scale, input_factor):
    # effective weight on host: rank-4 expansion + sign, fp16,
    # transposed + partition-major
    w_eff = np.sign(weight, dtype=np.float32) * (
        weight_scale.astype(np.float32) @ input_factor.astype(np.float32)
    )
    w16 = w_eff.astype(np.float16)  # [OUT, IN]
    xT = (
        x.reshape(T, IN)
        .T.astype(np.float16)
        .reshape(NBLK, P, T)
        .transpose(1, 0, 2)
        .reshape(P, NBLK * T)
    )
    xT = np.ascontiguousarray(xT)
    in_maps = []
    for c in range(NCORES):
        wc = w16[c * OS : (c + 1) * OS].T  # [IN, OS]
        wc = (
            wc.reshape(NBLK, P, OS)
            .transpose(1, 0, 2)
            .reshape(P, NBLK * OS)
        )
        in_maps.append(
            {"wT": np.ascontiguousarray(wc), "xT": xT}
        )
    return in_maps


def gather_out(results):
    outs = [results[c]["out"] for c in range(NCORES)]
    full = np.concatenate(outs, axis=1)  # [T, OUT] fp16
    return np.ascontiguousarray(full.reshape(B, S, OUT).astype(np.float32))


def kernel(x, weight, weight_scale, input_factor):
    global _NC_CACHE
    from concourse.bass_utils import run_bass_kernel_spmd

    if _NC_CACHE is None:
        _NC_CACHE = build_nc()
    nc = _NC_CACHE

    in_maps = make_in_maps(x, weight, weight_scale, input_factor)
    res = run_bass_kernel_spmd(nc, in_maps, core_ids=list(range(NCORES)))
    return gather_out(res.results)


if __name__ == "__main__":
    # quick self-run with random data
    rng = np.random.default_rng(0)
    x = rng.standard_normal((B, S, IN), dtype=np.float32)
    w = rng.standard_normal((OUT, IN), dtype=np.float32)
    ws = rng.standard_normal((OUT, R), dtype=np.float32)
    f = rng.standard_normal((R, IN), dtype=np.float32)
    out = kernel(x=x, weight=w, weight_scale=ws, input_factor=f)
    wv = ws @ f
    expected = np.einsum("bsi,oi->bso", x, np.sign(w) * wv)
    rel = np.abs(out - expected).max() / np.abs(expected).max()
    print("rel err:", rel)


# revision 5
# speedup vs baseline: 2.6234x; 1.0246x over previous
"""BitLinear kernel for Trainium2, 8-core column-parallel.

Computes out = x @ (sign(W) * (weight_scale @ input_factor)).T
  x: [32, 8, 4096] f32, W: [11008, 4096] f32,
  weight_scale: [11008, 4] f32, input_factor: [4, 4096] f32
  -> out: [32, 8, 11008] f32

Sharding: column-parallel over out_features (11008 = 8 x 1376). Each core
gets its effective-weight row-shard plus replicated x; host concatenates.
No collectives.

The effective weight w_eff = sign(W) * (weight_scale @ input_factor) is
formed on the host (one rank-4 expansion + sign multiply), cast to fp16
(~5e-4 rel err, well inside the 2e-2 gate) and shipped PRE-TRANSPOSED and
partition-major, so every device DMA descriptor is a contiguous 2.75-5.5 KB
line -- no DMA transpose, no on-device sign/value work, line-rate HBM.

Per-core dataflow:
  - wT [128, 32, 1376] fp16 streams as macro-tiles on the sync HWDGE ring
    (first two macros are single K-blocks so the first matmul unblocks
    early, then 2-block macros).
  - xT [128, 32, 256] fp16 resident in SBUF, 4 chunks on the scalar ring.
  - A short burst of dummy warm-up matmuls on a zeroed tile keeps the PE
    busy from t~=7.5us so HAM un-throttles (1.2 -> 2.4 GHz) before the
    first real matmul's data lands.
  - PE: for each of 32 K-blocks, 2 token-blocks x 3 N-chunks (512/512/352)
    accumulate out[t, o] += xT_blk.T @ wT_blk in PSUM (6 banks).
    Redundant per-chunk LDWEIGHTS are deduped post-build (one stationary
    load per (K-block, token-block) instead of three).
  - Epilogue: per-chunk ACT copies PSUM -> SBUF fp16, DMA out on the sync
    ring; host upcasts to f32.
"""

import sys

if "/opt/trn_rl_repo" not in sys.path:
    sys.path.insert(0, "/opt/trn_rl_repo")

import numpy as np

# ---------------------------------------------------------------------------
# problem constants (hardcoded per the self-contained-kernel contract)
B, S, IN, OUT, R = 32, 8, 4096, 11008, 4
T = B * S               # 256 tokens
NCORES = 8
OS = OUT // NCORES      # 1376 out-features per core
P = 128
NBLK = IN // P          # 32 K-blocks
O_CHUNKS = [(0, 512), (512, 512), (1024, 352)]
N_WARMUP = 14                       # dummy PE matmuls to warm the HAM

# Dual-ring DMA schedule: the W stream is split across both HWDGE rings
# (sync + scalar/act) because one ring tops out at ~280 GB/s while the
# HBM-per-core budget is ~358.  Each entry is (ring, kind, start, len):
#   ("w", k0, kl): W macro covering K-blocks [k0, k0+kl)
#   ("x", a0, al): xT chunk covering blocks [a0, a0+al)
# Emission order = per-ring FIFO order; W macros must appear in k order.
SYNC_Q = [
    ("w", 0, 1), ("w", 2, 2), ("w", 6, 2), ("x", 8, 8), ("w", 10, 2),
    ("w", 14, 2), ("x", 24, 8), ("w", 18, 2), ("w", 22, 2), ("w", 26, 2),
    ("w", 30, 2),
]
SCALAR_Q = [
    ("x", 0, 4), ("w", 1, 1), ("x", 4, 4), ("w", 4, 2), ("w", 8, 2),
    ("w", 12, 2), ("x", 16, 8), ("w", 16, 2), ("w", 20, 2), ("w", 24, 2),
    ("w", 28, 2),
]


def _install_tile_drain_patch():
    """This walrus build rejects >2 sync waits on one TPB_CTRL instruction;
    split the TileContext end-of-kernel drain into one drain per proc."""
    from concourse.tile import TileContext
    from concourse.vector_clock import ScopedClock
    from bass_rust import VectorClock

    if getattr(TileContext, "_drain_patch_installed", False):
        return

    def patched_drain_and_barrier(self, tick_clock, wait_clock):
        nc = self.nc
        gc = tick_clock.global_clock
        for i in range(27):
            v = gc[i]
            if v > 0:
                single = [0] * 27
                single[i] = v
                d = nc.sync.drain()
                wait_clock.add_sem_waits(
                    d.ins, ScopedClock({None: VectorClock(single)})
                )
        nc.all_engine_barrier()
        assert self.sems is not None
        popped = nc._tile_sem_poison_stack.pop()
        assert popped is self._sem_poison
        nc.clear_and_free_semaphores(list(self.sems.allocated().values()))
        nc.all_engine_barrier()

    TileContext._drain_and_barrier = patched_drain_and_barrier
    TileContext._drain_patch_installed = True


def _split_excess_waits(nc, max_waits=1):
    """This walrus build rejects instructions carrying more than ~2 sync
    waits. Move excess waits onto no-op instructions inserted immediately
    before the offender on the same engine (same semantics: the engine
    performs the same waits, in order, before executing the instruction)."""
    import concourse.mybir as mybir

    n_split = 0
    for fn in nc.m.functions:
        for bb in fn.blocks:
            insts = list(bb.instructions)
            new = []
            changed = False
            for inst in insts:
                si = inst.sync_info
                waits = list(si.on_wait) if si is not None else []
                if len(waits) > max_waits:
                    changed = True
                    n_split += 1
                    excess = waits[:-max_waits]
                    keep = waits[-max_waits:]
                    for i in range(0, len(excess), max_waits):
                        chunk = excess[i : i + max_waits]
                        nop = mybir.InstNoOp(
                            name=nc.get_next_instruction_name(),
                            sync_info=mybir.SyncInfo(
                                on_wait=chunk, on_update=[]
                            ),
                            bass_nofuse=True,
                            engine=inst.engine,
                        )
                        new.append(nop)
                    inst.sync_info = mybir.SyncInfo(
                        on_wait=keep, on_update=list(si.on_update)
                    )
                new.append(inst)
            if changed:
                bb.instructions = new
    return n_split


def _dedup_ldweights(nc):
    """Legalization splits every InstMatmult into LDWEIGHTS+MATMUL, so a
    stationary operand reused by consecutive matmuls (our 3 N-chunks per
    token-block) is reloaded each time. Drop an InstLdweights whose
    signature (access pattern, perf mode, tile position/size) matches the
    previous one on the PE queue with only matmuls/semaphores in between;
    its waits/updates migrate to the next PE instruction."""
    n_removed = 0
    passthrough = {"InstMatmult", "InstNoOp", "InstEventSemaphore", "InstDrain"}
    for fn in nc.m.functions:
        for bb in fn.blocks:
            insts = list(bb.instructions)
            new = []
            last_sig = None
            pend_waits = []
            pend_updates = []
            changed = False
            for inst in insts:
                tn = type(inst).__name__
                is_pe = getattr(inst, "engine", None) == nc.tensor.engine
                if tn == "InstLdweights" and is_pe:
                    ap = inst.ins[0]
                    sig = (
                        ap.concise(),
                        getattr(ap, "offset", None),
                        str(inst.perf_mode),
                        str(inst.is_transpose),
                        str(inst.tile_position),
                        str(inst.tile_size),
                    )
                    if sig == last_sig:
                        si = inst.sync_info
                        if si is not None:
                            pend_waits.extend(si.on_wait)
                            pend_updates.extend(si.on_update)
                        n_removed += 1
                        changed = True
                        continue
                    last_sig = sig
                elif is_pe and tn not in passthrough:
                    last_sig = None
                if is_pe and (pend_waits or pend_updates):
                    import concourse.mybir as mybir

                    si = inst.sync_info
                    waits = list(si.on_wait) if si is not None else []
                    updates = list(si.on_update) if si is not None else []
                    seen = {
                        (w.sync_type, w.id, w.wait_mode, w.wait_value)
                        for w in waits
                    }
                    for w in pend_waits:
                        k = (w.sync_type, w.id, w.wait_mode, w.wait_value)
                        if k not in seen:
                            seen.add(k)
                            waits.append(w)
                    updates.extend(pend_updates)
                    inst.sync_info = mybir.SyncInfo(
                        on_wait=waits, on_update=updates
                    )
                    pend_waits = []
                    pend_updates = []
                new.append(inst)
            if changed:
                bb.instructions = new
    return n_removed


def build_nc():
    import concourse.bass as bass
    import concourse.mybir as mybir
    from concourse.bass import ts
    from concourse.tile import TileContext

    _install_tile_drain_patch()

    F16 = mybir.dt.float16
    F32 = mybir.dt.float32
    nc = bass.Bass("TRN2", num_devices=NCORES)

    wT_ext = nc.dram_tensor(
        "wT", [P, NBLK * OS], F16, kind="ExternalInput"
    ).ap()
    xT_ext = nc.dram_tensor(
        "xT", [P, NBLK * T], F16, kind="ExternalInput"
    ).ap()
    out_ext = nc.dram_tensor("out", [T, OS], F16, kind="ExternalOutput").ap()

    with TileContext(nc) as tc:
        with (
            tc.tile_pool(name="const", bufs=1) as cpool,
            tc.tile_pool(name="wpool", bufs=4) as wpool,
            tc.tile_pool(name="outsb", bufs=2) as outsb,
            tc.tile_pool(name="opsum", bufs=2, space="PSUM") as opool,
            tc.tile_pool(name="wupsum", bufs=1, space="PSUM") as wupool,
        ):
            # --- PE warm-up: dummy matmuls on an iota-filled tile, no DMA
            # deps, so the HAM clock gate opens before real data lands.
            wu_a = cpool.tile([P, 256], F16)
            nc.gpsimd.iota(
                wu_a[:, :],
                [[1, 256]],
                channel_multiplier=0,
                allow_small_or_imprecise_dtypes=True,
            )
            wu_ps = wupool.tile([P, 256], F32)
            for _ in range(N_WARMUP):
                nc.tensor.matmul(
                    wu_ps, wu_a[:, :P], wu_a, start=True, stop=True
                )

            xT_sb = cpool.tile([P, NBLK, T], F16)
            xT_view = xT_ext.rearrange("p (a t) -> p a t", t=T)
            wT_view = wT_ext.rearrange("p (k o) -> p k o", o=OS)
            out_ps = [
                opool.tile([P, OS], F32, tag="out_ps", name=f"out_ps{tb}")
                for tb in range(2)
            ]

            rings = [
                (list(SYNC_Q), nc.sync),
                (list(SCALAR_Q), nc.scalar),
            ]
            w_tiles = {}  # k0 -> (tile, k0, kl)

            def emit_ring_until(k_needed):
                """Pop entries off both ring queues (in per-ring FIFO
                order) until the W macro covering k_needed is emitted."""
                while k_needed not in w_tiles:
                    for q, eng in rings:
                        if not q:
                            continue
                        kind, s0, sl = q[0]
                        if kind == "x":
                            q.pop(0)
                            eng.dma_start(
                                xT_sb[:, s0 : s0 + sl],
                                xT_view[:, s0 : s0 + sl],
                            )
                        elif s0 <= k_needed:
                            q.pop(0)
                            w_sb = wpool.tile(
                                [P, sl, OS], F16,
                                tag=f"w_sb{sl}", name="w_sb",
                            )
                            eng.dma_start(
                                w_sb[:, :, :], wT_view[:, s0 : s0 + sl]
                            )
                            for k in range(s0, s0 + sl):
                                w_tiles[k] = (w_sb, s0)

            for ib in range(NBLK):
                emit_ring_until(ib)
                w_sb, k0 = w_tiles[ib]
                first = ib == 0
                last = ib == NBLK - 1
                for tb in range(2):
                    for (o0, No) in O_CHUNKS:
                        nc.tensor.matmul(
                            out_ps[tb][:, o0 : o0 + No],
                            xT_sb[:, ib, ts(tb, P)],
                            w_sb[:, ib - k0, o0 : o0 + No],
                            start=first,
                            stop=last,
                        )

            # --- epilogue: PSUM -> SBUF fp16 on the DVE (fastest copier),
            # one DMA per token-block on the now-idle sync ring.
            for tb in range(2):
                o_sb = outsb.tile([P, OS], F16, tag="o_sb", name="o_sb")
                nc.vector.tensor_copy(o_sb, out_ps[tb])
                nc.sync.dma_start(out_ext[ts(tb, P), :], o_sb)

    _dedup_ldweights(nc)
    _split_excess_waits(nc)
    return nc


_NC_CACHE = None


def make_in_maps(x, weight, weight_scale, input_factor):
    # effective weight on host: rank-4 expansion + sign, fp16,
    # transposed + partition-major
    w_eff = np.sign(weight, dtype=np.float32) * (
        weight_scale.astype(np.float32) @ input_factor.astype(np.float32)
    )
    w16 = w_eff.astype(np.float16)  # [OUT, IN]
    xT = (
        x.reshape(T, IN)
        .T.astype(np.float16)
        .reshape(NBLK, P, T)
        .transpose(1, 0, 2)
        .reshape(P, NBLK * T)
    )
    xT = np.ascontiguousarray(xT)
    in_maps = []
    for c in range(NCORES):
        wc = w16[c * OS : (c + 1) * OS].T  # [IN, OS]
        wc = (
            wc.reshape(NBLK, P, OS)
            .transpose(1, 0, 2)
            .reshape(P, NBLK * OS)
        )
        in_maps.append(
            {"wT": np.ascontiguousarray(wc), "xT": xT}
        )
    return in_maps


def gather_out(results):
    outs = [results[c]["out"] for c in range(NCORES)]
    full = np.concatenate(outs, axis=1)  # [T, OUT] fp16
    return np.ascontiguousarray(full.reshape(B, S, OUT).astype(np.float32))


def kernel(x, weight, weight_scale, input_factor):
    global _NC_CACHE
    from concourse.bass_utils import run_bass_kernel_spmd

    if _NC_CACHE is None:
        _NC_CACHE = build_nc()
    nc = _NC_CACHE

    in_maps = make_in_maps(x, weight, weight_scale, input_factor)
    res = run_bass_kernel_spmd(nc, in_maps, core_ids=list(range(NCORES)))
    return gather_out(res.results)


if __name__ == "__main__":
    # quick self-run with random data
    rng = np.random.default_rng(0)
    x = rng.standard_normal((B, S, IN), dtype=np.float32)
    w = rng.standard_normal((OUT, IN), dtype=np.float32)
    ws = rng.standard_normal((OUT, R), dtype=np.float32)
    f = rng.standard_normal((R, IN), dtype=np.float32)
    out = kernel(x=x, weight=w, weight_scale=ws, input_factor=f)
    wv = ws @ f
    expected = np.einsum("bsi,oi->bso", x, np.sign(w) * wv)
    rel = np.abs(out - expected).max() / np.abs(expected).max()
    print("rel err:", rel)
